# revision 15
# baseline (speedup 1.0000x reference)
"""BM25 scoring kernel for Trainium2 (8 NeuronCores, SPMD) — v4, routed.

score = sum_v term1(qtf_v) * term2(ptf_v) * term3(dfs_v)

Only vocab ids present in the query contribute (term1=0 elsewhere), so we
work query-position-centric:

  score = sum_i term2(ptf[t_i]) * term3(dfs[t_i]) / (K3 + qtf[t_i])

Sharding (the "route ids to owning shard" strategy): the host splits the
vocab into 128 ranges of 2^16 ids (bucket = id >> 16) and assigns whole
buckets to (core, partition-group-of-8) slots: a load-balancing pass
spreads the heavy buckets across cores, and within each core buckets are
ordered by descending query-occupancy.  All query positions and passage
ids of a bucket are routed to its group, so counting needs only
within-group comparisons:

  - ptf: 8 count units compare each slot column [128,1] against the
    group's passage list [128,PCAP] (bucket p-ids replicated across the
    group's 8 partitions), split between DVE (tensor_scalar not_equal
    with accum) and ACT (Sign+Square with accum) — both count NON-matches
    into halves of one accumulator tile (memset to PCAP so the merge
    `S = praw_d + praw_i` is uniform; ptf = 2*PCAP - S).
  - qtf: bucket query ids are sorted and laid out column-PAIR-major, so
    duplicates sit adjacent within a row pair; qtf = 1 + matches against
    2 host-staged shifted copies (one eq + one reduce).  Exact for ids
    repeated <= 2 times (host asserts; P ~ 1e-7 per seed).
  - dfs: per-core table slice (the core's 16 bucket blocks concatenated
    in group order, 2^20 entries); occupied slot columns are gathered by
    per-column indirect (SWDGE) DMAs.  The descending-occupancy group
    order makes later columns live in a partition PREFIX, so their DMAs
    carry fewer descriptors (SWDGE emission is ~9ns/descriptor); DMAs
    are emitted largest-first so the smallest one's transfer tail gates
    the Ln that consumes the gather.

All pads use values that can never equal a real id or another compared
pad, so pad slots get ptf=0 => exactly zero contribution (no masks).
The per-core partial is reduced on-chip (PE matmul against ones) and the
8 scalars are summed on the host (the final sum all-reduce).
"""

import math
import os
from contextlib import ExitStack

import numpy as np

import concourse.bacc as bacc
import concourse.bass as bass
import concourse.tile as tile
from concourse import mybir
from concourse.bass_utils import run_bass_kernel_spmd

# ---- problem constants (from the BM25 reference) ----
VOCAB = 8_388_608
NQ = 4096
NP = 8192
K1, K3, B = 1.2, 8.0, 0.75
N_DOCS = 8_841_823.0
L_AVE = 55.0
L_D = NP
C2 = K1 * (1.0 - B + B * L_D / L_AVE)  # term2 denominator constant
INV_LN2 = 1.0 / math.log(2.0)

NCORES = 8
P = 128
NBUCK = 128              # global buckets: id >> 16
GSHIFT = 16
GROUPS = 16              # buckets per core
G = 8                    # partitions per bucket group
KQ = 8                   # slot columns (bucket capacity 64)
PCAP = 96                # passage ids per bucket (key(0) max 84; asserted)
GCOLS = 6                # slot columns that may hold real ids (asserted)
BSIZE = 1 << GSHIFT      # ids per bucket

# fp32-exact constants for the exact-zero property of term2 at ptf=0
K1L32 = np.float32(K1 * INV_LN2)
PTF_OFF = np.float32(np.float32(2 * PCAP) * K1L32)

# which ptf count units run on ACT (Sign+Square); rest on DVE
ACT_UNITS = tuple(
    int(x) for x in os.environ.get("BM25_ACT_UNITS", "4,5,6,7").split(",") if x != ""
)

F32 = mybir.dt.float32
I32 = mybir.dt.int32

# A-tensor column layout
A_MQ2 = 0                  # myq repeated 2x   [0, 16)
A_SH = 16                  # sh1|shm1          [16, 32)
A_NEG = 32                 # -myq              [32, 40)
A_ONE = 40                 # ones              [40, 41)
A_GIX = 41                 # gather offsets (int32 bits as f32) [41, 41+GCOLS)
A_PL = A_GIX + GCOLS       # plist             [.., ..+PCAP)
A_W = A_PL + PCAP


def _build_program(npart):
    """npart[k]: partitions to gather for slot column k (0 => skip)."""
    nc = bacc.Bacc(
        "TRN2", target_bir_lowering=False, debug=False, num_devices=NCORES
    )
    A = nc.dram_tensor("A", [P, A_W], F32, kind="ExternalInput").ap()
    dfss = nc.dram_tensor("dfss", [1 << 20, 1], F32, kind="ExternalInput").ap()
    partial = nc.dram_tensor("partial", [1, 1], F32, kind="ExternalOutput").ap()

    with tile.TileContext(nc) as tc, ExitStack() as ctx:
        pool = ctx.enter_context(tc.tile_pool(name="main", bufs=1))
        gpool = ctx.enter_context(tc.tile_pool(name="sgn", bufs=3))
        dpool = ctx.enter_context(tc.tile_pool(name="dummy", bufs=2))
        ppool = ctx.enter_context(tc.tile_pool(name="psum", bufs=1, space="PSUM"))

        # init tiles (DVE) — gpsimd's first op must be the first gather
        bias_a = pool.tile([P, 1], F32)
        nc.vector.memset(bias_a[:], float(N_DOCS + 0.5))
        bias_b = pool.tile([P, 1], F32)
        nc.vector.memset(bias_b[:], 0.5)
        praw2 = pool.tile([P, 2 * KQ], F32)
        nc.vector.memset(praw2[:], float(PCAP))
        dfsg = pool.tile([P, KQ], F32)
        nc.vector.memset(dfsg[:], 500.0)
        praw_d = praw2[:, 0:KQ]
        praw_i = praw2[:, KQ : 2 * KQ]

        # single input DMA
        At = pool.tile([P, A_W], F32)
        nc.sync.dma_start(out=At[:], in_=A[:])

        myq2 = At[:, A_MQ2 : A_MQ2 + 2 * KQ]
        sh2 = At[:, A_SH : A_SH + 2 * KQ]
        myq = At[:, A_MQ2 : A_MQ2 + KQ]
        negq = At[:, A_NEG : A_NEG + KQ]
        ones = At[:, A_ONE : A_ONE + 1]
        gixt = At[:, A_GIX : A_GIX + GCOLS].bitcast(I32)
        plist = At[:, A_PL : A_PL + PCAP]

        # dfs gathers: largest first so the smallest transfer tail is last
        order = sorted(
            [k for k in range(GCOLS) if npart[k] > 0],
            key=lambda k: -npart[k],
        )
        for k in order:
            n = npart[k]
            nc.gpsimd.indirect_dma_start(
                out=dfsg[0:n, k : k + 1],
                out_offset=None,
                in_=dfss[:],
                in_offset=bass.IndirectOffsetOnAxis(
                    ap=gixt[0:n, k : k + 1], axis=0
                ),
            )

        # ACT: first op loads the Ln table set (overlaps the DMAs)
        warm = pool.tile([P, 1], F32)
        nc.scalar.activation(
            warm[:], bias_a[:], mybir.ActivationFunctionType.Ln,
            bias=bias_b[:], scale=1.0,
        )

        # ---- qtf from the shifted copies: one eq + one reduce ----
        eq = pool.tile([P, 2 * KQ], F32)
        nc.vector.tensor_tensor(eq[:], myq2, sh2, mybir.AluOpType.is_equal)
        qtfs = pool.tile([P, KQ], F32)
        nc.vector.tensor_reduce(
            out=qtfs[:],
            in_=eq[:].rearrange("p (b k) -> p k b", b=2),
            axis=mybir.AxisListType.X,
            op=mybir.AluOpType.add,
        )
        acc = pool.tile([P, KQ], F32)  # K3 + qtf (qtf = 1 + shift matches)
        nc.vector.tensor_scalar(
            out=acc[:], in0=qtfs[:], scalar1=float(K3 + 1.0), scalar2=None,
            op0=mybir.AluOpType.add,
        )
        rac = pool.tile([P, KQ], F32)
        nc.vector.reciprocal(rac[:], acc[:])

        # ---- ptf count units (inverted: count NON-matches) ----
        for k in range(KQ):
            if k in ACT_UNITS:
                sgn = gpool.tile([P, PCAP], F32, tag="sgn")
                nc.scalar.activation(
                    sgn[:], plist, mybir.ActivationFunctionType.Sign,
                    bias=negq[:, k : k + 1], scale=1.0,
                )
                dmy = dpool.tile([P, PCAP], F32, tag="dmy")
                nc.scalar.activation(
                    dmy[:], sgn[:], mybir.ActivationFunctionType.Square,
                    bias=0.0, scale=1.0, accum_out=praw_i[:, k : k + 1],
                )
            else:
                dmy = dpool.tile([P, PCAP], F32, tag="dmy")
                nc.vector.tensor_scalar(
                    out=dmy[:], in0=plist, scalar1=myq[:, k : k + 1],
                    scalar2=None, op0=mybir.AluOpType.not_equal,
                    op1=mybir.AluOpType.add,
                    accum_out=praw_d[:, k : k + 1],
                )

        # ---- merge + term2 ----
        S = pool.tile([P, KQ], F32)  # S = 2*PCAP - ptf
        nc.vector.tensor_add(S[:], praw_d, praw_i)
        ptfK = pool.tile([P, KQ], F32)  # K1/ln2 * ptf (exact 0 at ptf=0)
        nc.vector.tensor_scalar(
            out=ptfK[:], in0=S[:], scalar1=float(-K1L32), scalar2=float(PTF_OFF),
            op0=mybir.AluOpType.mult, op1=mybir.AluOpType.add,
        )
        den = pool.tile([P, KQ], F32)  # ptf + C2
        nc.vector.tensor_scalar(
            out=den[:], in0=S[:], scalar1=-1.0, scalar2=float(2 * PCAP + C2),
            op0=mybir.AluOpType.mult, op1=mybir.AluOpType.add,
        )
        rden = pool.tile([P, KQ], F32)
        nc.vector.reciprocal(rden[:], den[:])
        t2 = pool.tile([P, KQ], F32)
        nc.vector.tensor_mul(t2[:], ptfK[:], rden[:])
        v = pool.tile([P, KQ], F32)
        nc.vector.tensor_mul(v[:], t2[:], rac[:])

        # ---- term3 = ln(N+0.5 - dfs) - ln(dfs + 0.5) ----
        la = pool.tile([P, KQ], F32)
        nc.scalar.activation(
            la[:], dfsg[:], mybir.ActivationFunctionType.Ln,
            bias=bias_a[:], scale=-1.0,
        )
        lb = pool.tile([P, KQ], F32)
        nc.scalar.activation(
            lb[:], dfsg[:], mybir.ActivationFunctionType.Ln,
            bias=bias_b[:], scale=1.0,
        )
        t3 = pool.tile([P, KQ], F32)
        nc.vector.tensor_sub(t3[:], la[:], lb[:])

        # ---- w = v * t3, row-sum, partition reduce via PE ----
        w = pool.tile([P, KQ], F32)
        nc.vector.tensor_mul(w[:], v[:], t3[:])
        rowsum = pool.tile([P, 1], F32)
        nc.vector.tensor_reduce(
            out=rowsum[:], in_=w[:],
            axis=mybir.AxisListType.X, op=mybir.AluOpType.add,
        )
        pacc = ppool.tile([1, 1], F32, space="PSUM")
        nc.tensor.matmul(pacc[:], lhsT=rowsum[:], rhs=ones, start=True, stop=True)
        res = pool.tile([1, 1], F32)
        nc.vector.tensor_copy(res[:], pacc[:])
        nc.sync.dma_start(out=partial[:], in_=res[:])

    nc.compile()
    return nc


_NC_CACHE = {}


def _get_program(npart):
    key = tuple(npart)
    if key not in _NC_CACHE:
        _NC_CACHE[key] = _build_program(key)
    return _NC_CACHE[key]


def _layout_bucket_q(ids_sorted):
    """Column-pair-major placement (see module docstring).  Returns
    [(row 0..G-1, col 0..KQ-1, id)]."""
    out = []
    pos = 0
    i = 0
    n = len(ids_sorted)
    while i < n:
        run = 1
        while i + run < n and ids_sorted[i + run] == ids_sorted[i]:
            run += 1
        assert run <= 2, f"query id repeated {run} times; widen shift window"
        if run == 2 and pos % 2 == 1:
            pos += 1  # align the pair horizontally
        for t in range(run):
            q, r = divmod(pos + t, 2)
            pair, row = divmod(q, G)
            col = pair * 2 + r
            assert col < GCOLS, "bucket overflows GCOLS slot columns"
            out.append((row, col, ids_sorted[i + t]))
        pos += run
        i += run
    return out


def _assign_buckets(qcounts):
    """Spread buckets over cores balanced by query-count (greedy on the
    descending sequence), then order each core's buckets descending.
    Returns list per core of bucket ids (length GROUPS)."""
    order = sorted(range(NBUCK), key=lambda b: -qcounts[b])
    loads = [0.0] * NCORES
    slots = [[] for _ in range(NCORES)]
    for b in order:
        c = min(
            (c for c in range(NCORES) if len(slots[c]) < GROUPS),
            key=lambda c: loads[c],
        )
        slots[c].append(b)
        loads[c] += qcounts[b]
    return slots  # already descending per core


def make_in_maps(query_ids, passage_ids, dfs):
    q = np.asarray(query_ids).reshape(-1).astype(np.int64)
    p = np.asarray(passage_ids).reshape(-1).astype(np.int64)
    d = np.ascontiguousarray(np.asarray(dfs, dtype=np.float32).reshape(-1, 1))
    qb = (q >> GSHIFT).astype(np.int64)
    pb = (p >> GSHIFT).astype(np.int64)
    qcounts = np.bincount(qb, minlength=NBUCK)
    cores = _assign_buckets(qcounts)

    in_maps = []
    used = np.zeros((NCORES, P, KQ), bool)
    for c in range(NCORES):
        # unique pad values per slot (never equal a real id or another pad)
        myq = -(4.0 + np.arange(P * KQ, dtype=np.float64)).reshape(P, KQ)
        plist = np.full((P, PCAP), -99999.0, np.float64)
        gixm = np.zeros((P, GCOLS), np.int32)
        for j, b in enumerate(cores[c]):
            qsel = np.sort(q[qb == b])
            for row, col, val in _layout_bucket_q(qsel):
                prow = j * G + row
                myq[prow, col] = float(val)
                gixm[prow, col] = j * BSIZE + (int(val) & (BSIZE - 1))
                used[c, prow, col] = True
            psel = p[pb == b]
            assert psel.size <= PCAP, f"passage bucket overflow {psel.size}"
            plist[j * G : j * G + G, : psel.size] = psel.astype(np.float64)

        BIG = -1.0e9
        sh1 = np.full((P, KQ), BIG)
        sh1[:, 1::2] = myq[:, 0::2]
        sm1 = np.full((P, KQ), BIG)
        sm1[:, 0::2] = myq[:, 1::2]

        A = np.empty((P, A_W), np.float32)
        A[:, A_MQ2 : A_MQ2 + KQ] = myq
        A[:, A_MQ2 + KQ : A_MQ2 + 2 * KQ] = myq
        A[:, A_SH : A_SH + KQ] = sh1
        A[:, A_SH + KQ : A_SH + 2 * KQ] = sm1
        A[:, A_NEG : A_NEG + KQ] = -myq
        A[:, A_ONE] = 1.0
        A[:, A_GIX : A_GIX + GCOLS] = gixm.view(np.float32)
        A[:, A_PL : A_PL + PCAP] = plist

        dfs_c = np.concatenate(
            [d[b * BSIZE : (b + 1) * BSIZE] for b in cores[c]], axis=0
        )
        in_maps.append({"A": A, "dfss": np.ascontiguousarray(dfs_c)})

    # partitions to gather per column: max over cores of highest used row+1
    npart = []
    for k in range(GCOLS):
        m = 0
        for c in range(NCORES):
            rows = np.nonzero(used[c, :, k])[0]
            if rows.size:
                m = max(m, int(rows[-1]) + 1)
        npart.append(m)
    return in_maps, npart


def kernel(query_ids, passage_ids, dfs, **run_kwargs):
    in_maps, npart = make_in_maps(query_ids, passage_ids, dfs)
    nc = _get_program(npart)
    res = run_bass_kernel_spmd(nc, in_maps, core_ids=list(range(NCORES)), **run_kwargs)
    total = np.float32(sum(float(r["partial"][0, 0]) for r in res.results))
    out = np.array([total], dtype=np.float32)
    kernel.last_results = res
    return out


# revision 16
# speedup vs baseline: 1.0032x; 1.0032x over previous
"""BM25 scoring kernel for Trainium2 (8 NeuronCores, SPMD) — v4, routed.

score = sum_v term1(qtf_v) * term2(ptf_v) * term3(dfs_v)

Only vocab ids present in the query contribute (term1=0 elsewhere), so we
work query-position-centric:

  score = sum_i term2(ptf[t_i]) * term3(dfs[t_i]) / (K3 + qtf[t_i])

Sharding (the "route ids to owning shard" strategy): the host splits the
vocab into 128 ranges of 2^16 ids (bucket = id >> 16) and assigns whole
buckets to (core, partition-group-of-8) slots: a load-balancing pass
spreads the heavy buckets across cores, and within each core buckets are
ordered by descending query-occupancy.  All query positions and passage
ids of a bucket are routed to its group, so counting needs only
within-group comparisons:

  - ptf: 8 count units compare each slot column [128,1] against the
    group's passage list [128,PCAP] (bucket p-ids replicated across the
    group's 8 partitions), split between DVE (tensor_scalar not_equal
    with accum) and ACT (Sign+Square with accum) — both count NON-matches
    into halves of one accumulator tile (memset to PCAP so the merge
    `S = praw_d + praw_i` is uniform; ptf = 2*PCAP - S).
  - qtf: bucket query ids are sorted and laid out column-PAIR-major, so
    duplicates sit adjacent within a row pair; qtf = 1 + matches against
    2 host-staged shifted copies (one eq + one reduce).  Exact for ids
    repeated <= 2 times (host asserts; P ~ 1e-7 per seed).
  - dfs: per-core table slice (the core's 16 bucket blocks concatenated
    in group order, 2^20 entries); occupied slot columns are gathered by
    per-column indirect (SWDGE) DMAs.  The descending-occupancy group
    order makes later columns live in a partition PREFIX, so their DMAs
    carry fewer descriptors (SWDGE emission is ~9ns/descriptor); DMAs
    are emitted largest-first so the smallest one's transfer tail gates
    the Ln that consumes the gather.

All pads use values that can never equal a real id or another compared
pad, so pad slots get ptf=0 => exactly zero contribution (no masks).
The per-core partial is reduced on-chip (PE matmul against ones) and the
8 scalars are summed on the host (the final sum all-reduce).
"""

import math
import os
from contextlib import ExitStack

import numpy as np

import concourse.bacc as bacc
import concourse.bass as bass
import concourse.tile as tile
from concourse import mybir
from concourse.bass_utils import run_bass_kernel_spmd

# ---- problem constants (from the BM25 reference) ----
VOCAB = 8_388_608
NQ = 4096
NP = 8192
K1, K3, B = 1.2, 8.0, 0.75
N_DOCS = 8_841_823.0
L_AVE = 55.0
L_D = NP
C2 = K1 * (1.0 - B + B * L_D / L_AVE)  # term2 denominator constant
INV_LN2 = 1.0 / math.log(2.0)

NCORES = 8
P = 128
NBUCK = 128              # global buckets: id >> 16
GSHIFT = 16
GROUPS = 16              # buckets per core
G = 8                    # partitions per bucket group
KQ = 8                   # slot columns (bucket capacity 64)
PCAP = 96                # passage ids per bucket (key(0) max 84; asserted)
GCOLS = 6                # slot columns that may hold real ids (asserted)
BSIZE = 1 << GSHIFT      # ids per bucket

# fp32-exact constants for the exact-zero property of term2 at ptf=0
K1L32 = np.float32(K1 * INV_LN2)
PTF_OFF = np.float32(np.float32(2 * PCAP) * K1L32)

# which ptf count units run on ACT (Sign+Square); rest on DVE
ACT_UNITS = tuple(
    int(x) for x in os.environ.get("BM25_ACT_UNITS", "4,5,6,7").split(",") if x != ""
)

F32 = mybir.dt.float32
I32 = mybir.dt.int32

# A-tensor column layout
A_MQ2 = 0                  # myq repeated 2x   [0, 16)
A_SH = 16                  # sh1|shm1          [16, 32)
A_NEG = 32                 # -myq              [32, 40)
A_ONE = 40                 # ones              [40, 41)
A_GIX = 41                 # gather offsets (int32 bits as f32) [41, 41+GCOLS)
A_PL = A_GIX + GCOLS       # plist             [.., ..+PCAP)
A_W = A_PL + PCAP


def _build_program(npart):
    """npart[k]: partitions to gather for slot column k (0 => skip)."""
    nc = bacc.Bacc(
        "TRN2", target_bir_lowering=False, debug=False, num_devices=NCORES
    )
    A = nc.dram_tensor("A", [P, A_W], F32, kind="ExternalInput").ap()
    gix = nc.dram_tensor("gix", [P, GCOLS], I32, kind="ExternalInput").ap()
    dfss = nc.dram_tensor("dfss", [1 << 20, 1], F32, kind="ExternalInput").ap()
    partial = nc.dram_tensor("partial", [1, 1], F32, kind="ExternalOutput").ap()

    with tile.TileContext(nc) as tc, ExitStack() as ctx:
        pool = ctx.enter_context(tc.tile_pool(name="main", bufs=1))
        gpool = ctx.enter_context(tc.tile_pool(name="sgn", bufs=3))
        dpool = ctx.enter_context(tc.tile_pool(name="dummy", bufs=2))
        ppool = ctx.enter_context(tc.tile_pool(name="psum", bufs=1, space="PSUM"))

        # init tiles (DVE) — gpsimd's first op must be the first gather
        bias_a = pool.tile([P, 1], F32)
        nc.vector.memset(bias_a[:], float(N_DOCS + 0.5))
        bias_b = pool.tile([P, 1], F32)
        nc.vector.memset(bias_b[:], 0.5)
        praw2 = pool.tile([P, 2 * KQ], F32)
        nc.vector.memset(praw2[:], float(PCAP))
        dfsg = pool.tile([P, KQ], F32)
        nc.vector.memset(dfsg[:], 500.0)
        praw_d = praw2[:, 0:KQ]
        praw_i = praw2[:, KQ : 2 * KQ]

        # small offsets DMA first (its completion gates the gathers)
        gixt = pool.tile([P, GCOLS], I32)
        nc.sync.dma_start(out=gixt[:], in_=gix[:])
        At = pool.tile([P, A_W], F32)
        nc.sync.dma_start(out=At[:], in_=A[:])

        myq2 = At[:, A_MQ2 : A_MQ2 + 2 * KQ]
        sh2 = At[:, A_SH : A_SH + 2 * KQ]
        myq = At[:, A_MQ2 : A_MQ2 + KQ]
        negq = At[:, A_NEG : A_NEG + KQ]
        ones = At[:, A_ONE : A_ONE + 1]
        plist = At[:, A_PL : A_PL + PCAP]

        # dfs gathers: largest first so the smallest transfer tail is last
        order = sorted(
            [k for k in range(GCOLS) if npart[k] > 0],
            key=lambda k: -npart[k],
        )
        for k in order:
            n = npart[k]
            nc.gpsimd.indirect_dma_start(
                out=dfsg[0:n, k : k + 1],
                out_offset=None,
                in_=dfss[:],
                in_offset=bass.IndirectOffsetOnAxis(
                    ap=gixt[0:n, k : k + 1], axis=0
                ),
            )

        # ACT: first op loads the Ln table set (overlaps the DMAs)
        warm = pool.tile([P, 1], F32)
        nc.scalar.activation(
            warm[:], bias_a[:], mybir.ActivationFunctionType.Ln,
            bias=bias_b[:], scale=1.0,
        )

        # ---- qtf from the shifted copies: one eq + one reduce ----
        eq = pool.tile([P, 2 * KQ], F32)
        nc.vector.tensor_tensor(eq[:], myq2, sh2, mybir.AluOpType.is_equal)
        qtfs = pool.tile([P, KQ], F32)
        nc.vector.tensor_reduce(
            out=qtfs[:],
            in_=eq[:].rearrange("p (b k) -> p k b", b=2),
            axis=mybir.AxisListType.X,
            op=mybir.AluOpType.add,
        )
        acc = pool.tile([P, KQ], F32)  # K3 + qtf (qtf = 1 + shift matches)
        nc.vector.tensor_scalar(
            out=acc[:], in0=qtfs[:], scalar1=float(K3 + 1.0), scalar2=None,
            op0=mybir.AluOpType.add,
        )
        rac = pool.tile([P, KQ], F32)
        nc.vector.reciprocal(rac[:], acc[:])

        # ---- ptf count units (inverted: count NON-matches) ----
        for k in range(KQ):
            if k in ACT_UNITS:
                sgn = gpool.tile([P, PCAP], F32, tag="sgn")
                nc.scalar.activation(
                    sgn[:], plist, mybir.ActivationFunctionType.Sign,
                    bias=negq[:, k : k + 1], scale=1.0,
                )
                dmy = dpool.tile([P, PCAP], F32, tag="dmy")
                nc.scalar.activation(
                    dmy[:], sgn[:], mybir.ActivationFunctionType.Square,
                    bias=0.0, scale=1.0, accum_out=praw_i[:, k : k + 1],
                )
            else:
                dmy = dpool.tile([P, PCAP], F32, tag="dmy")
                nc.vector.tensor_scalar(
                    out=dmy[:], in0=plist, scalar1=myq[:, k : k + 1],
                    scalar2=None, op0=mybir.AluOpType.not_equal,
                    op1=mybir.AluOpType.add,
                    accum_out=praw_d[:, k : k + 1],
                )

        # ---- merge + term2 ----
        S = pool.tile([P, KQ], F32)  # S = 2*PCAP - ptf
        nc.vector.tensor_add(S[:], praw_d, praw_i)
        ptfK = pool.tile([P, KQ], F32)  # K1/ln2 * ptf (exact 0 at ptf=0)
        nc.vector.tensor_scalar(
            out=ptfK[:], in0=S[:], scalar1=float(-K1L32), scalar2=float(PTF_OFF),
            op0=mybir.AluOpType.mult, op1=mybir.AluOpType.add,
        )
        den = pool.tile([P, KQ], F32)  # ptf + C2
        nc.vector.tensor_scalar(
            out=den[:], in0=S[:], scalar1=-1.0, scalar2=float(2 * PCAP + C2),
            op0=mybir.AluOpType.mult, op1=mybir.AluOpType.add,
        )
        rden = pool.tile([P, KQ], F32)
        nc.vector.reciprocal(rden[:], den[:])
        t2 = pool.tile([P, KQ], F32)
        nc.vector.tensor_mul(t2[:], ptfK[:], rden[:])
        v = pool.tile([P, KQ], F32)
        nc.vector.tensor_mul(v[:], t2[:], rac[:])

        # ---- term3 = ln(N+0.5 - dfs) - ln(dfs + 0.5) ----
        la = pool.tile([P, KQ], F32)
        nc.scalar.activation(
            la[:], dfsg[:], mybir.ActivationFunctionType.Ln,
            bias=bias_a[:], scale=-1.0,
        )
        lb = pool.tile([P, KQ], F32)
        nc.scalar.activation(
            lb[:], dfsg[:], mybir.ActivationFunctionType.Ln,
            bias=bias_b[:], scale=1.0,
        )
        t3 = pool.tile([P, KQ], F32)
        nc.vector.tensor_sub(t3[:], la[:], lb[:])

        # ---- w = v * t3, row-sum, partition reduce via PE ----
        w = pool.tile([P, KQ], F32)
        nc.vector.tensor_mul(w[:], v[:], t3[:])
        rowsum = pool.tile([P, 1], F32)
        nc.vector.tensor_reduce(
            out=rowsum[:], in_=w[:],
            axis=mybir.AxisListType.X, op=mybir.AluOpType.add,
        )
        pacc = ppool.tile([1, 1], F32, space="PSUM")
        nc.tensor.matmul(pacc[:], lhsT=rowsum[:], rhs=ones, start=True, stop=True)
        res = pool.tile([1, 1], F32)
        nc.vector.tensor_copy(res[:], pacc[:])
        nc.sync.dma_start(out=partial[:], in_=res[:])

    nc.compile()
    return nc


_NC_CACHE = {}


def _get_program(npart):
    key = tuple(npart)
    if key not in _NC_CACHE:
        _NC_CACHE[key] = _build_program(key)
    return _NC_CACHE[key]


def _layout_bucket_q(ids_sorted):
    """Column-pair-major placement (see module docstring).  Returns
    [(row 0..G-1, col 0..KQ-1, id)]."""
    out = []
    pos = 0
    i = 0
    n = len(ids_sorted)
    while i < n:
        run = 1
        while i + run < n and ids_sorted[i + run] == ids_sorted[i]:
            run += 1
        assert run <= 2, f"query id repeated {run} times; widen shift window"
        if run == 2 and pos % 2 == 1:
            pos += 1  # align the pair horizontally
        for t in range(run):
            q, r = divmod(pos + t, 2)
            pair, row = divmod(q, G)
            col = pair * 2 + r
            assert col < GCOLS, "bucket overflows GCOLS slot columns"
            out.append((row, col, ids_sorted[i + t]))
        pos += run
        i += run
    return out


def _assign_buckets(qcounts):
    """Spread buckets over cores balanced by query-count (greedy on the
    descending sequence), then order each core's buckets descending.
    Returns list per core of bucket ids (length GROUPS)."""
    order = sorted(range(NBUCK), key=lambda b: -qcounts[b])
    loads = [0.0] * NCORES
    slots = [[] for _ in range(NCORES)]
    for b in order:
        c = min(
            (c for c in range(NCORES) if len(slots[c]) < GROUPS),
            key=lambda c: loads[c],
        )
        slots[c].append(b)
        loads[c] += qcounts[b]
    return slots  # already descending per core


def make_in_maps(query_ids, passage_ids, dfs):
    q = np.asarray(query_ids).reshape(-1).astype(np.int64)
    p = np.asarray(passage_ids).reshape(-1).astype(np.int64)
    d = np.ascontiguousarray(np.asarray(dfs, dtype=np.float32).reshape(-1, 1))
    qb = (q >> GSHIFT).astype(np.int64)
    pb = (p >> GSHIFT).astype(np.int64)
    qcounts = np.bincount(qb, minlength=NBUCK)
    cores = _assign_buckets(qcounts)

    in_maps = []
    used = np.zeros((NCORES, P, KQ), bool)
    for c in range(NCORES):
        # unique pad values per slot (never equal a real id or another pad)
        myq = -(4.0 + np.arange(P * KQ, dtype=np.float64)).reshape(P, KQ)
        plist = np.full((P, PCAP), -99999.0, np.float64)
        gixm = np.zeros((P, GCOLS), np.int32)
        for j, b in enumerate(cores[c]):
            qsel = np.sort(q[qb == b])
            for row, col, val in _layout_bucket_q(qsel):
                prow = j * G + row
                myq[prow, col] = float(val)
                gixm[prow, col] = j * BSIZE + (int(val) & (BSIZE - 1))
                used[c, prow, col] = True
            psel = p[pb == b]
            assert psel.size <= PCAP, f"passage bucket overflow {psel.size}"
            plist[j * G : j * G + G, : psel.size] = psel.astype(np.float64)

        BIG = -1.0e9
        sh1 = np.full((P, KQ), BIG)
        sh1[:, 1::2] = myq[:, 0::2]
        sm1 = np.full((P, KQ), BIG)
        sm1[:, 0::2] = myq[:, 1::2]

        A = np.empty((P, A_W), np.float32)
        A[:, A_MQ2 : A_MQ2 + KQ] = myq
        A[:, A_MQ2 + KQ : A_MQ2 + 2 * KQ] = myq
        A[:, A_SH : A_SH + KQ] = sh1
        A[:, A_SH + KQ : A_SH + 2 * KQ] = sm1
        A[:, A_NEG : A_NEG + KQ] = -myq
        A[:, A_ONE] = 1.0
        A[:, A_GIX : A_GIX + GCOLS] = gixm.view(np.float32)
        A[:, A_PL : A_PL + PCAP] = plist

        dfs_c = np.concatenate(
            [d[b * BSIZE : (b + 1) * BSIZE] for b in cores[c]], axis=0
        )
        in_maps.append({
            "A": A, "gix": np.ascontiguousarray(gixm),
            "dfss": np.ascontiguousarray(dfs_c),
        })

    # partitions to gather per column: max over cores of highest used row+1
    npart = []
    for k in range(GCOLS):
        m = 0
        for c in range(NCORES):
            rows = np.nonzero(used[c, :, k])[0]
            if rows.size:
                m = max(m, int(rows[-1]) + 1)
        npart.append(m)
    return in_maps, npart


def kernel(query_ids, passage_ids, dfs, **run_kwargs):
    in_maps, npart = make_in_maps(query_ids, passage_ids, dfs)
    nc = _get_program(npart)
    res = run_bass_kernel_spmd(nc, in_maps, core_ids=list(range(NCORES)), **run_kwargs)
    total = np.float32(sum(float(r["partial"][0, 0]) for r in res.results))
    out = np.array([total], dtype=np.float32)
    kernel.last_results = res
    return out


# revision 17
# speedup vs baseline: 1.2065x; 1.2027x over previous
"""BM25 scoring kernel for Trainium2 (8 NeuronCores, SPMD) — v4, routed.

score = sum_v term1(qtf_v) * term2(ptf_v) * term3(dfs_v)

Only vocab ids present in the query contribute (term1=0 elsewhere), so we
work query-position-centric:

  score = sum_i term2(ptf[t_i]) * term3(dfs[t_i]) / (K3 + qtf[t_i])

Sharding (the "route ids to owning shard" strategy): the host splits the
vocab into 128 ranges of 2^16 ids (bucket = id >> 16) and assigns whole
buckets to (core, partition-group-of-8) slots: a load-balancing pass
spreads the heavy buckets across cores, and within each core buckets are
ordered by descending query-occupancy.  All query positions and passage
ids of a bucket are routed to its group, so counting needs only
within-group comparisons:

  - ptf: 8 count units compare each slot column [128,1] against the
    group's passage list [128,PCAP] (bucket p-ids replicated across the
    group's 8 partitions), split between DVE (tensor_scalar not_equal
    with accum) and ACT (Sign+Square with accum) — both count NON-matches
    into halves of one accumulator tile (memset to PCAP so the merge
    `S = praw_d + praw_i` is uniform; ptf = 2*PCAP - S).
  - qtf: bucket query ids are sorted and laid out column-PAIR-major, so
    duplicates sit adjacent within a row pair; qtf = 1 + matches against
    2 host-staged shifted copies (one eq + one reduce).  Exact for ids
    repeated <= 2 times (host asserts; P ~ 1e-7 per seed).
  - dfs: per-core table slice (the core's 16 bucket blocks concatenated
    in group order, 2^20 entries); occupied slot columns are gathered by
    per-column indirect (SWDGE) DMAs.  The descending-occupancy group
    order makes later columns live in a partition PREFIX, so their DMAs
    carry fewer descriptors (SWDGE emission is ~9ns/descriptor); DMAs
    are emitted largest-first so the smallest one's transfer tail gates
    the Ln that consumes the gather.

All pads use values that can never equal a real id or another compared
pad, so pad slots get ptf=0 => exactly zero contribution (no masks).
The per-core partial is reduced on-chip (PE matmul against ones) and the
8 scalars are summed on the host (the final sum all-reduce).
"""

import math
import os
from contextlib import ExitStack

import numpy as np

import concourse.bacc as bacc
import concourse.bass as bass
import concourse.tile as tile
from concourse import mybir
from concourse.bass_utils import run_bass_kernel_spmd

# ---- problem constants (from the BM25 reference) ----
VOCAB = 8_388_608
NQ = 4096
NP = 8192
K1, K3, B = 1.2, 8.0, 0.75
N_DOCS = 8_841_823.0
L_AVE = 55.0
L_D = NP
C2 = K1 * (1.0 - B + B * L_D / L_AVE)  # term2 denominator constant
INV_LN2 = 1.0 / math.log(2.0)

NCORES = 8
P = 128
NBUCK = 128              # global buckets: id >> 16
GSHIFT = 16
GROUPS = 16              # buckets per core
G = 8                    # partitions per bucket group
KQ = 8                   # slot columns (bucket capacity 64)
PCAP = 96                # passage ids per bucket (key(0) max 84; asserted)
GCOLS = 6                # slot columns that may hold real ids (asserted)
BSIZE = 1 << GSHIFT      # ids per bucket

# fp32-exact constants for the exact-zero property of term2 at ptf=0
K1L32 = np.float32(K1 * INV_LN2)
PTF_OFF = np.float32(np.float32(2 * PCAP) * K1L32)

# which ptf count units run on ACT (Sign+Square); rest on DVE
ACT_UNITS = tuple(
    int(x) for x in os.environ.get("BM25_ACT_UNITS", "4,5,6,7").split(",") if x != ""
)

F32 = mybir.dt.float32
I32 = mybir.dt.int32

# A-tensor column layout
A_MQ2 = 0                  # myq repeated 2x   [0, 16)
A_SH = 16                  # sh1|shm1          [16, 32)
A_NEG = 32                 # -myq              [32, 40)
A_ONE = 40                 # ones              [40, 41)
A_GIX = 41                 # gather offsets (int32 bits as f32) [41, 41+GCOLS)
A_PL = A_GIX + GCOLS       # plist             [.., ..+PCAP)
A_W = A_PL + PCAP


def _build_program(npart):
    """npart[k]: partitions to gather for slot column k (0 => skip)."""
    nc = bacc.Bacc(
        "TRN2", target_bir_lowering=False, debug=False, num_devices=NCORES
    )
    A = nc.dram_tensor("A", [P, A_W], F32, kind="ExternalInput").ap()
    gix = nc.dram_tensor("gix", [P, GCOLS], I32, kind="ExternalInput").ap()
    dfss = nc.dram_tensor("dfss", [1 << 20, 1], F32, kind="ExternalInput").ap()
    partial = nc.dram_tensor("partial", [1, 1], F32, kind="ExternalOutput").ap()

    with tile.TileContext(nc) as tc, ExitStack() as ctx:
        pool = ctx.enter_context(tc.tile_pool(name="main", bufs=1))
        gpool = ctx.enter_context(tc.tile_pool(name="sgn", bufs=3))
        dpool = ctx.enter_context(tc.tile_pool(name="dummy", bufs=2))
        ppool = ctx.enter_context(tc.tile_pool(name="psum", bufs=1, space="PSUM"))

        # init tiles (DVE) — gpsimd's first op must be the first gather
        bias_a = pool.tile([P, 1], F32)
        nc.vector.memset(bias_a[:], float(N_DOCS + 0.5))
        bias_b = pool.tile([P, 1], F32)
        nc.vector.memset(bias_b[:], 0.5)
        praw2 = pool.tile([P, 2 * KQ], F32)
        nc.vector.memset(praw2[:], float(PCAP))
        dfsg = pool.tile([P, KQ], F32)
        nc.vector.memset(dfsg[:], 500.0)
        praw_d = praw2[:, 0:KQ]
        praw_i = praw2[:, KQ : 2 * KQ]

        # small offsets DMA first (its completion gates the gathers)
        gixt = pool.tile([P, GCOLS], I32)
        nc.sync.dma_start(out=gixt[:], in_=gix[:])
        At = pool.tile([P, A_W], F32)
        nc.sync.dma_start(out=At[:], in_=A[:])

        myq2 = At[:, A_MQ2 : A_MQ2 + 2 * KQ]
        sh2 = At[:, A_SH : A_SH + 2 * KQ]
        myq = At[:, A_MQ2 : A_MQ2 + KQ]
        negq = At[:, A_NEG : A_NEG + KQ]
        ones = At[:, A_ONE : A_ONE + 1]
        plist = At[:, A_PL : A_PL + PCAP]

        # dfs gathers: largest first so the smallest transfer tail is last
        order = sorted(
            [k for k in range(GCOLS) if npart[k] > 0],
            key=lambda k: -npart[k],
        )
        for k in order:
            n = npart[k]
            nc.gpsimd.indirect_dma_start(
                out=dfsg[0:n, k : k + 1],
                out_offset=None,
                in_=dfss[:],
                in_offset=bass.IndirectOffsetOnAxis(
                    ap=gixt[0:n, k : k + 1], axis=0
                ),
            )

        # ACT: first op loads the Ln table set (overlaps the DMAs)
        warm = pool.tile([P, 1], F32)
        nc.scalar.activation(
            warm[:], bias_a[:], mybir.ActivationFunctionType.Ln,
            bias=bias_b[:], scale=1.0,
        )

        # ---- qtf from the shifted copies: one eq + one reduce ----
        eq = pool.tile([P, 2 * KQ], F32)
        nc.vector.tensor_tensor(eq[:], myq2, sh2, mybir.AluOpType.is_equal)
        qtfs = pool.tile([P, KQ], F32)
        nc.vector.tensor_reduce(
            out=qtfs[:],
            in_=eq[:].rearrange("p (b k) -> p k b", b=2),
            axis=mybir.AxisListType.X,
            op=mybir.AluOpType.add,
        )
        acc = pool.tile([P, KQ], F32)  # K3 + qtf (qtf = 1 + shift matches)
        nc.vector.tensor_scalar(
            out=acc[:], in0=qtfs[:], scalar1=float(K3 + 1.0), scalar2=None,
            op0=mybir.AluOpType.add,
        )
        rac = pool.tile([P, KQ], F32)
        nc.vector.reciprocal(rac[:], acc[:])

        # ---- ptf count units (inverted: count NON-matches) ----
        for k in range(KQ):
            if k in ACT_UNITS:
                sgn = gpool.tile([P, PCAP], F32, tag="sgn")
                nc.scalar.activation(
                    sgn[:], plist, mybir.ActivationFunctionType.Sign,
                    bias=negq[:, k : k + 1], scale=1.0,
                )
                dmy = dpool.tile([P, PCAP], F32, tag="dmy")
                nc.scalar.activation(
                    dmy[:], sgn[:], mybir.ActivationFunctionType.Square,
                    bias=0.0, scale=1.0, accum_out=praw_i[:, k : k + 1],
                )
            else:
                dmy = dpool.tile([P, PCAP], F32, tag="dmy")
                nc.vector.tensor_scalar(
                    out=dmy[:], in0=plist, scalar1=myq[:, k : k + 1],
                    scalar2=None, op0=mybir.AluOpType.not_equal,
                    op1=mybir.AluOpType.add,
                    accum_out=praw_d[:, k : k + 1],
                )

        # ---- merge + term2 ----
        S = pool.tile([P, KQ], F32)  # S = 2*PCAP - ptf
        nc.vector.tensor_add(S[:], praw_d, praw_i)
        ptfK = pool.tile([P, KQ], F32)  # K1/ln2 * ptf (exact 0 at ptf=0)
        nc.vector.tensor_scalar(
            out=ptfK[:], in0=S[:], scalar1=float(-K1L32), scalar2=float(PTF_OFF),
            op0=mybir.AluOpType.mult, op1=mybir.AluOpType.add,
        )
        den = pool.tile([P, KQ], F32)  # ptf + C2
        nc.vector.tensor_scalar(
            out=den[:], in0=S[:], scalar1=-1.0, scalar2=float(2 * PCAP + C2),
            op0=mybir.AluOpType.mult, op1=mybir.AluOpType.add,
        )
        rden = pool.tile([P, KQ], F32)
        nc.vector.reciprocal(rden[:], den[:])
        t2 = pool.tile([P, KQ], F32)
        nc.vector.tensor_mul(t2[:], ptfK[:], rden[:])
        v = pool.tile([P, KQ], F32)
        nc.vector.tensor_mul(v[:], t2[:], rac[:])

        # ---- term3 = ln(N+0.5 - dfs) - ln(dfs + 0.5) ----
        la = pool.tile([P, KQ], F32)
        nc.scalar.activation(
            la[:], dfsg[:], mybir.ActivationFunctionType.Ln,
            bias=bias_a[:], scale=-1.0,
        )
        lb = pool.tile([P, KQ], F32)
        nc.scalar.activation(
            lb[:], dfsg[:], mybir.ActivationFunctionType.Ln,
            bias=bias_b[:], scale=1.0,
        )
        t3 = pool.tile([P, KQ], F32)
        nc.vector.tensor_sub(t3[:], la[:], lb[:])

        # ---- w = v * t3, row-sum, partition reduce via PE ----
        w = pool.tile([P, KQ], F32)
        nc.vector.tensor_mul(w[:], v[:], t3[:])
        rowsum = pool.tile([P, 1], F32)
        nc.vector.tensor_reduce(
            out=rowsum[:], in_=w[:],
            axis=mybir.AxisListType.X, op=mybir.AluOpType.add,
        )
        pacc = ppool.tile([1, 1], F32, space="PSUM")
        nc.tensor.matmul(pacc[:], lhsT=rowsum[:], rhs=ones, start=True, stop=True)
        res = pool.tile([1, 1], F32)
        nc.vector.tensor_copy(res[:], pacc[:])
        nc.sync.dma_start(out=partial[:], in_=res[:])

    nc.compile()
    return nc


_NC_CACHE = {}


def _get_program(npart):
    key = tuple(npart)
    if key not in _NC_CACHE:
        _NC_CACHE[key] = _build_program(key)
    return _NC_CACHE[key]


def _layout_bucket_q(ids_sorted):
    """Column-pair-major placement (see module docstring).  Returns
    [(row 0..G-1, col 0..KQ-1, id)]."""
    out = []
    pos = 0
    i = 0
    n = len(ids_sorted)
    while i < n:
        run = 1
        while i + run < n and ids_sorted[i + run] == ids_sorted[i]:
            run += 1
        assert run <= 2, f"query id repeated {run} times; widen shift window"
        if run == 2 and pos % 2 == 1:
            pos += 1  # align the pair horizontally
        for t in range(run):
            q, r = divmod(pos + t, 2)
            pair, row = divmod(q, G)
            col = pair * 2 + r
            assert col < GCOLS, "bucket overflows GCOLS slot columns"
            out.append((row, col, ids_sorted[i + t]))
        pos += run
        i += run
    return out


def _assign_buckets(qcounts):
    """Spread buckets over cores balanced by query-count (greedy on the
    descending sequence), then order each core's buckets descending.
    Returns list per core of bucket ids (length GROUPS)."""
    order = sorted(range(NBUCK), key=lambda b: -qcounts[b])
    loads = [0.0] * NCORES
    slots = [[] for _ in range(NCORES)]
    for b in order:
        c = min(
            (c for c in range(NCORES) if len(slots[c]) < GROUPS),
            key=lambda c: loads[c],
        )
        slots[c].append(b)
        loads[c] += qcounts[b]
    return slots  # already descending per core


def make_in_maps(query_ids, passage_ids, dfs):
    q = np.asarray(query_ids).reshape(-1).astype(np.int64)
    p = np.asarray(passage_ids).reshape(-1).astype(np.int64)
    d = np.ascontiguousarray(np.asarray(dfs, dtype=np.float32).reshape(-1, 1))
    qb = (q >> GSHIFT).astype(np.int64)
    pb = (p >> GSHIFT).astype(np.int64)
    qcounts = np.bincount(qb, minlength=NBUCK)
    cores = _assign_buckets(qcounts)

    in_maps = []
    used = np.zeros((NCORES, P, KQ), bool)
    for c in range(NCORES):
        # unique pad values per slot (never equal a real id or another pad)
        myq = -(4.0 + np.arange(P * KQ, dtype=np.float64)).reshape(P, KQ)
        plist = np.full((P, PCAP), -99999.0, np.float64)
        gixm = np.zeros((P, GCOLS), np.int32)
        for j, b in enumerate(cores[c]):
            qsel = np.sort(q[qb == b])
            for row, col, val in _layout_bucket_q(qsel):
                prow = j * G + row
                myq[prow, col] = float(val)
                gixm[prow, col] = j * BSIZE + (int(val) & (BSIZE - 1))
                used[c, prow, col] = True
            psel = p[pb == b]
            assert psel.size <= PCAP, f"passage bucket overflow {psel.size}"
            plist[j * G : j * G + G, : psel.size] = psel.astype(np.float64)

        BIG = -1.0e9
        sh1 = np.full((P, KQ), BIG)
        sh1[:, 1::2] = myq[:, 0::2]
        sm1 = np.full((P, KQ), BIG)
        sm1[:, 0::2] = myq[:, 1::2]

        A = np.empty((P, A_W), np.float32)
        A[:, A_MQ2 : A_MQ2 + KQ] = myq
        A[:, A_MQ2 + KQ : A_MQ2 + 2 * KQ] = myq
        A[:, A_SH : A_SH + KQ] = sh1
        A[:, A_SH + KQ : A_SH + 2 * KQ] = sm1
        A[:, A_NEG : A_NEG + KQ] = -myq
        A[:, A_ONE] = 1.0
        A[:, A_GIX : A_GIX + GCOLS] = gixm.view(np.float32)
        A[:, A_PL : A_PL + PCAP] = plist

        dfs_c = np.concatenate(
            [d[b * BSIZE : (b + 1) * BSIZE] for b in cores[c]], axis=0
        )
        in_maps.append({
            "A": A, "gix": np.ascontiguousarray(gixm),
            "dfss": np.ascontiguousarray(dfs_c),
        })

    # partitions to gather per column: full width except the last column
    # (slicing adds fixed emission cost; only the final DMA's small size
    # matters, to keep its transfer tail short)
    m = 0
    for c in range(NCORES):
        rows = np.nonzero(used[c, :, GCOLS - 1])[0]
        if rows.size:
            m = max(m, int(rows[-1]) + 1)
    npart = [P] * (GCOLS - 1) + [max(m, 8)]
    return in_maps, npart


def kernel(query_ids, passage_ids, dfs, **run_kwargs):
    in_maps, npart = make_in_maps(query_ids, passage_ids, dfs)
    nc = _get_program(npart)
    res = run_bass_kernel_spmd(nc, in_maps, core_ids=list(range(NCORES)), **run_kwargs)
    total = np.float32(sum(float(r["partial"][0, 0]) for r in res.results))
    out = np.array([total], dtype=np.float32)
    kernel.last_results = res
    return out
